# revision 25
# baseline (speedup 1.0000x reference)
"""Extract-last-valid-token kernel for Trainium2 (raw Bass), 8-core SPMD.

Computation (per batch row b):
    idx_b = max(sum(attention_mask[b]) - 1, 0)
    out[b] = decoder_outputs[b, idx_b, :]

The reference implements this as a one-hot multiply-reduce over the full
[B, S, H] tensor (256 MiB of reads).  Here each core instead reads only its
mask shard (64 KiB), computes the per-row index on-chip, and gathers the 4
needed rows (2 KiB each) with an indirect DMA — the memory-optimal algorithm.

Sharding: pure data-parallel over the batch dim (B=32 -> 4 rows per core),
no cross-core communication.

Written in raw Bass (no Tile framework) to avoid Tile's all-engine entry
barrier, kernel-tail drain and per-semaphore clear loop, which dominated the
runtime of the Tile version (18.5 us vs ~5 us of actual work).  Same-engine
DVE chains are ordered by the per-op pipeline DRAIN; cross-engine edges use
manual semaphores.

Index pipeline (all f32 until the final int32 cast; sums <= 4096 are exact):
  reduce mask [32p, 4b, 128] -> partial [32p, 4b]      (DVE, int32)
  cast -> partial_f                                    (DVE)
  matmul ones[32,1].T @ partial_f -> sums [1, 4] PSUM  (PE)
  relu(sums - 1) + row_offsets -> idxg_f [1, 4]        (DVE)
  matmul idxg_f[1,4].T @ one1[1,1] -> [4, 1] PSUM      (PE, transposes)
  cast -> idxg int32 [4, 1]                            (DVE)
  indirect gather rows[4, 512] from DRAM               (GpSimd SWDGE)
  store rows -> out                                    (Sync HWDGE)
"""

import os
import sys
from contextlib import ExitStack

import numpy as np

for _p in ("/opt/trn_rl_repo",):
    if os.path.isdir(_p) and _p not in sys.path:
        sys.path.insert(0, _p)

B, S, H = 32, 4096, 512
N_CORES = 8
BS = B // N_CORES          # batch rows per core
PCHUNK = 32                # partitions used for the mask layout
FCHUNK = S // PCHUNK       # 128 contiguous elements (512 B) per DMA run

_nc_cache = None


def _build_nc():
    """Build the single-core Bass program (same program runs on all 8 cores)."""
    import concourse.bass as bass
    from concourse import mybir

    nc = bass.Bass("TRN2", target_bir_lowering=False, debug=False)
    # Same-engine DVE chains are ordered by the per-op pipeline DRAIN on HW;
    # CoreSim's race detector doesn't model that, so quiet it for sim runs.
    nc.detect_race_conditions = False

    do = nc.dram_tensor(
        "decoder_outputs", [BS, S, H], mybir.dt.float32, kind="ExternalInput"
    ).ap()
    am = nc.dram_tensor(
        "attention_mask", [BS, S], mybir.dt.int32, kind="ExternalInput"
    ).ap()
    out = nc.dram_tensor(
        "out", [BS, H], mybir.dt.float32, kind="ExternalOutput"
    ).ap()

    i32 = mybir.dt.int32
    f32 = mybir.dt.float32

    with ExitStack() as ctx:
        ec = ctx.enter_context
        mask_i = ec(nc.sbuf_tensor([PCHUNK, BS * FCHUNK], i32))
        partial_i = ec(nc.sbuf_tensor([PCHUNK, BS], i32))
        partial_f = ec(nc.sbuf_tensor([PCHUNK, BS], f32))
        ones = ec(nc.sbuf_tensor([PCHUNK, 1], f32))
        one1 = ec(nc.sbuf_tensor([1, 1], f32))
        offs = ec(nc.sbuf_tensor([1, BS], f32))
        idxf = ec(nc.sbuf_tensor([1, BS], f32))
        idxg_f = ec(nc.sbuf_tensor([1, BS], f32))
        idxg = ec(nc.sbuf_tensor([BS, 1], i32))
        rows = ec(nc.sbuf_tensor([BS, H], f32))
        sums_ps = ec(nc.psum_tensor([1, BS], f32))
        idxg_ps = ec(nc.psum_tensor([BS, 1], f32))

        dma_sem = ec(nc.semaphore("dma_sem"))
        v_sem = ec(nc.semaphore("v_sem"))
        p_sem = ec(nc.semaphore("p_sem"))
        g_sem = ec(nc.semaphore("g_sem"))
        o_sem = ec(nc.semaphore("o_sem"))

        block = ec(nc.Block())

        @block.sync
        def _(sync: bass.BassEngine):
            # Mask shard [BS, S] laid out as [PCHUNK, BS, FCHUNK]: partition p
            # holds, for each row b, the contiguous 512 B run b*S + p*FCHUNK.
            sync.dma_start(
                out=mask_i[:].rearrange("p (b f) -> p b f", b=BS),
                in_=am.rearrange("b (p f) -> p b f", p=PCHUNK),
            ).then_inc(dma_sem, 16)
            # Store the gathered rows once the indirect DMA lands.
            sync.wait_ge(g_sem, 16)
            sync.dma_start(out=out[:, :], in_=rows[:, :]).then_inc(o_sem, 16)
            sync.wait_ge(o_sem, 16)
            # Leave all semaphores at 0 for potential NEFF re-execution.
            sync.sem_clear(dma_sem)
            sync.sem_clear(v_sem)
            sync.sem_clear(p_sem)
            sync.sem_clear(g_sem)
            sync.sem_clear(o_sem)

        @block.vector
        def _(vector: bass.BassEngine):
            # Constants: matmul ones-vectors and the row offsets b*S, all in
            # disjoint start-partition-0 regions.
            nc.vector.memset(ones[:, :], 1.0)
            nc.vector.memset(one1[:, :], 1.0)
            for b in range(BS):
                nc.vector.memset(offs[0:1, b : b + 1], float(b * S))

            vector.wait_ge(dma_sem, 16)
            # Segmented reduce over the free dim -> partial[p, b]
            with nc.allow_low_precision(
                reason="int32 accumulation of 0/1 mask is exact"
            ):
                nc.vector.reduce_sum(
                    out=partial_i[:, :],
                    in_=mask_i[:].rearrange("p (b f) -> p b f", b=BS),
                    axis=mybir.AxisListType.X,
                )
            # Back-to-back same-engine RAW needs an explicit pipeline drain in
            # raw Bass (Tile normally inserts these): without it the next op
            # reads stale SBUF.
            vector.drain()
            nc.vector.tensor_copy(out=partial_f[:, :], in_=partial_i[:, :])
            # Cross-engine signals must drain the write pipeline first — a
            # bare then_inc fires at retire, before the data is visible.
            vector.drain().then_inc(v_sem)

            # idx = max(sum - 1, 0) + b*S, on sums [1, BS] from the PE
            vector.wait_ge(p_sem, 1)
            nc.vector.tensor_scalar(
                out=idxf[:, :],
                in0=sums_ps[:, :],
                scalar1=-1.0,
                scalar2=0.0,
                op0=mybir.AluOpType.add,
                op1=mybir.AluOpType.max,
            )
            vector.drain()
            nc.vector.tensor_tensor(
                out=idxg_f[:, :], in0=idxf[:, :], in1=offs[:, :],
                op=mybir.AluOpType.add,
            )
            vector.drain().then_inc(v_sem)

            # Transposed index vector comes back from the PE; cast to int32.
            vector.wait_ge(p_sem, 2)
            nc.vector.tensor_copy(out=idxg[:, :], in_=idxg_ps[:, :])
            vector.drain().then_inc(v_sem)

        @block.tensor
        def _(tensor: bass.BassEngine):
            # Cross-partition sum: sums[0, b] = sum_p partial_f[p, b]
            tensor.wait_ge(v_sem, 1)
            nc.tensor.matmul(
                out=sums_ps[:, :], lhsT=ones[:, :], rhs=partial_f[:, :],
                start=True, stop=True,
            )
            tensor.drain().then_inc(p_sem)
            # Transpose [1, BS] -> [BS, 1] so the DGE sees one index per
            # partition.
            tensor.wait_ge(v_sem, 2)
            nc.tensor.matmul(
                out=idxg_ps[:, :], lhsT=idxg_f[:, :], rhs=one1[:, :],
                start=True, stop=True,
            )
            tensor.drain().then_inc(p_sem)

        @block.gpsimd
        def _(gpsimd: bass.BassEngine):
            gpsimd.wait_ge(v_sem, 3)
            # Gather the BS selected rows (H floats each) from DRAM.
            gpsimd.indirect_dma_start(
                out=rows[:, :],
                out_offset=None,
                in_=do.rearrange("b s h -> (b s) h"),
                in_offset=bass.IndirectOffsetOnAxis(ap=idxg[:, :1], axis=0),
            ).then_inc(g_sem, 16)

    return nc


def build_nc():
    global _nc_cache
    if _nc_cache is None:
        _nc_cache = _build_nc()
    return _nc_cache


def kernel(decoder_outputs, attention_mask):
    from concourse.bass_utils import run_bass_kernel_spmd

    decoder_outputs = np.ascontiguousarray(
        np.asarray(decoder_outputs, dtype=np.float32)
    )
    attention_mask = np.ascontiguousarray(np.asarray(attention_mask, dtype=np.int32))
    assert decoder_outputs.shape == (B, S, H)
    assert attention_mask.shape == (B, S)

    nc = build_nc()
    in_maps = [
        {
            "decoder_outputs": decoder_outputs[i * BS : (i + 1) * BS],
            "attention_mask": attention_mask[i * BS : (i + 1) * BS],
        }
        for i in range(N_CORES)
    ]
    res = run_bass_kernel_spmd(nc, in_maps, list(range(N_CORES)))
    return np.concatenate(
        [res.results[i]["out"] for i in range(N_CORES)], axis=0
    ).astype(np.float32)
